# revision 25
# baseline (speedup 1.0000x reference)
"""GQA attention block (Wq/Wk/Wv -> RoPE -> softmax(QK^T)V -> Wo) on 8 Trainium2
NeuronCores.

Sharding (tensor-parallel per the head-sharding scheme):
  core c in 0..7: batch b = c // 4, head-group g = c % 4.
  Each core owns 8 q-heads (global 8g..8g+7) and 2 kv-heads (2g, 2g+1) of one
  batch element, computes its slice of q/k/v projections, RoPE, attention, and
  a partial o_proj (Wo rows for its heads). The all-reduce after o_proj is the
  host-side unshard: out[b] = sum of the 4 partial outputs of batch b.

On-device layout (per core), everything feature-on-partitions ("transposed"):
  xt    [D=2048, S=2048]   x^T for this batch
  QT    [E=512,  S]        q^T; partition-tile j holds head pair (j, j+4):
                           local head j (kv0) on partitions 0:64, head j+4
                           (kv1) on partitions 64:128. Wq columns are permuted
                           on the host to produce this layout directly.
  KT    [128, S]           k^T; kv0 on partitions 0:64, kv1 on 64:128.
  V     [S, 130] as 16 tiles [128, 130]: cols 0:64 v(kv0), col 64 ones,
                           cols 65:129 v(kv1), col 129 ones  (v_aug).
  scores^T per head: [sk, sq] so exp is ACT psum->sbuf and the attn@v
  contraction (over sk) uses v_aug as the stationary operand; row 64 of the
  attn@v output is the softmax denominator (ones column trick).

Schedule (v2): phase 1 only computes q pairs 0,1 of chunk 0 and all K/V;
the remaining 14 Q-projection units run as PE filler inside the phase-2
attention pipeline (which is otherwise exp/ACT-gated), alongside the o_proj
chunks. The softmax 1/den broadcast runs on the idle GpSimd engine instead
of PE matmuls. Output is written bf16 (summed fp32 on host).
"""

import sys

if "/opt/trn_rl_repo" not in sys.path:
    sys.path.insert(0, "/opt/trn_rl_repo")

from contextlib import ExitStack

import numpy as np
import ml_dtypes

import concourse.bass as bass  # noqa: F401  (engine types via nc)
import concourse.tile as tile
from concourse import bacc, bass_utils, mybir

F32 = mybir.dt.float32
F32R = mybir.dt.float32r
BF16 = mybir.dt.bfloat16
AF = mybir.ActivationFunctionType

# Problem constants (hardcoded per harness contract)
B = 2
S = 2048  # sequence length
D = 2048  # d_model
N_HEADS = 32
N_KV = 8
HD = 64  # head dim
ROPE_BASE = 500000.0
N_CORES = 8

# Per-core derived
NQ = N_HEADS // 4  # 8 local q heads (4 head-groups)
E = NQ * HD  # 512 local q features
NPAIR = NQ // 2  # 4 head pairs / e-tiles
KVW = 2 * HD  # 128 local kv features
SC = 512  # s-chunk (projection + sq chunk)
NSC = S // SC  # 4
DT = D // 128  # 16 d-tiles
SKT = S // 128  # 16 sk tiles
ET = E // 128  # 4 e-tiles
SCALE = 1.0 / float(np.sqrt(HD))

SHUF_MASK = [(i + 16) % 32 for i in range(32)]


def build_program():
    nc = bacc.Bacc(
        "TRN2", target_bir_lowering=False, debug=False, enable_asserts=False
    )

    # All large inputs are pre-arranged on the host so each DMA reads one
    # contiguous run per partition (128 fat descriptors instead of 2048
    # small ones; descriptor generation on the issuing queue is the startup
    # bottleneck otherwise).
    xt = nc.dram_tensor("xt", [128, NSC, DT, SC], BF16, kind="ExternalInput").ap()
    wq = nc.dram_tensor("wq", [128, NPAIR, DT, 128], BF16, kind="ExternalInput").ap()
    wk = nc.dram_tensor("wk", [128, DT, KVW], BF16, kind="ExternalInput").ap()
    wv = nc.dram_tensor("wv", [128, DT, KVW], BF16, kind="ExternalInput").ap()
    wo = nc.dram_tensor("wo", [128, ET, D], BF16, kind="ExternalInput").ap()
    cosd = nc.dram_tensor("cosd", [128, S], BF16, kind="ExternalInput").ap()
    sind = nc.dram_tensor("sind", [128, S], BF16, kind="ExternalInput").ap()
    ident = nc.dram_tensor("ident", [128, 128], F32, kind="ExternalInput").ap()
    onesc = nc.dram_tensor("onesc", [128, 1], BF16, kind="ExternalInput").ap()
    out = nc.dram_tensor("out", [S, D], BF16, kind="ExternalOutput").ap()

    with tile.TileContext(nc) as tc, ExitStack() as ctx:
        persist = ctx.enter_context(tc.tile_pool(name="persist", bufs=1))
        xtp = ctx.enter_context(tc.tile_pool(name="xtp", bufs=3))
        ropec = ctx.enter_context(tc.tile_pool(name="ropec", bufs=1))
        ropet = ctx.enter_context(tc.tile_pool(name="ropet", bufs=2))

        # Persistent SBUF state
        qt_sb = [persist.tile([128, S], BF16, tag=f"qt{j}", name=f"qt{j}") for j in range(NPAIR)]
        kt_sb = persist.tile([128, S], BF16, tag="kt")
        v_sb = [persist.tile([128, 130], BF16, tag=f"v{j}", name=f"v{j}") for j in range(SKT)]
        attn_sb = [persist.tile([128, S], BF16, tag=f"at{j}", name=f"at{j}") for j in range(NPAIR)]
        onesc_sb = persist.tile([128, 1], BF16, tag="onesc")

        wq_sb = persist.tile([128, NPAIR, DT, 128], BF16, tag="wq")
        wk_sb = persist.tile([128, DT, KVW], BF16, tag="wk")
        wv_sb = persist.tile([128, DT, KVW], BF16, tag="wv")
        wo_sb = persist.tile([128, ET, D], BF16, tag="wo")

        # ---- input DMAs, spread across hw queues in first-use order ----
        # Per-queue DMA bandwidth is ~170GB/s with ~8us ring spin-up, so the
        # startup-critical loads (wq+xt0 for q-proj, then wk/wv/rope tables)
        # go on four separate queues. cos/sin load in per-chunk slices so
        # chunk-0 rope isn't gated on the full 4MB.
        xt_c = [None] * NSC
        for c in range(3):
            xt_c[c] = xtp.tile([128, DT, SC], BF16, tag="xt", name=f"xt_c{c}")
        # sync queue: xt chunk 0 in 4 t-group slices (q-proj mms start as
        # soon as the first slice + wq pair 0 land), then chunks 1 and (mid
        # phase-1) 3.
        for i in range(4):
            nc.sync.dma_start(
                out=xt_c[0][:, bass.ds(4 * i, 4), :],
                in_=xt[:, 0, bass.ds(4 * i, 4), :],
            )
        nc.sync.dma_start(out=xt_c[1], in_=xt[:, 1])
        # scalar queue: wq in pair slices (paced with the q-proj units),
        # small weights, xt chunk 2, then late rope tables.
        cos_sb = ropec.tile([128, S], BF16, tag="cos")
        sin_sb = ropec.tile([128, S], BF16, tag="sin")
        for j in range(NPAIR):
            nc.scalar.dma_start(out=wq_sb[:, j], in_=wq[:, j])
        nc.scalar.dma_start(out=wk_sb, in_=wk)
        nc.scalar.dma_start(out=wv_sb, in_=wv)
        ident_sb = ropec.tile([128, 128], F32, tag="ident")
        nc.scalar.dma_start(out=ident_sb, in_=ident)
        nc.scalar.dma_start(out=xt_c[2], in_=xt[:, 2])
        for c in range(2, NSC):
            nc.scalar.dma_start(out=cos_sb[:, bass.ts(c, SC)], in_=cosd[:, bass.ts(c, SC)])
            nc.scalar.dma_start(out=sin_sb[:, bass.ts(c, SC)], in_=sind[:, bass.ts(c, SC)])
        # gpsimd SWDGE: early rope tables (small; q/K rope of chunks 0-1
        # needs them by ~25us) + wo (needed ~+150us).
        nc.gpsimd.dma_start(out=onesc_sb, in_=onesc)
        for c in (0, 1):
            nc.gpsimd.dma_start(out=cos_sb[:, bass.ts(c, SC)], in_=cosd[:, bass.ts(c, SC)])
            nc.gpsimd.dma_start(out=sin_sb[:, bass.ts(c, SC)], in_=sind[:, bass.ts(c, SC)])
        nc.gpsimd.dma_start(out=wo_sb, in_=wo)

        # v_aug ones columns are constant: prefill once on gpsimd.
        for t in range(SKT):
            nc.gpsimd.tensor_copy(v_sb[t][:, 64:65], onesc_sb)
            nc.gpsimd.tensor_copy(v_sb[t][:, 129:130], onesc_sb)

        def rope(dst, src_ps, cs):
            """dst[:, cs*SC:+SC] = src_ps*cos + shuffle(src)*sin_signed.

            Features are laid out (host-side permutation) so the RoPE
            rotate pairing is a +-16 swap within each 32-partition
            quadrant; the rotate sign is folded into sind."""
            sl = bass.ts(cs, SC)
            raw = ropet.tile([128, SC], F32R, tag="raw", name="raw", bufs=3)
            nc.vector.tensor_copy(raw, src_ps)
            rp = ropet.tile([128, SC], F32, tag="shuf", name="shuf", bufs=3)
            nc.vector.stream_shuffle(rp, raw, SHUF_MASK)
            tcos = ropet.tile([128, SC], F32, tag="tmp", name="tcos", bufs=4)
            nc.vector.tensor_mul(tcos, raw, cos_sb[:, sl])
            tsin = ropet.tile([128, SC], F32, tag="tmp", name="tsin", bufs=4)
            nc.vector.tensor_mul(tsin, rp, sin_sb[:, sl])
            nc.vector.tensor_add(dst[:, sl], tcos, tsin)

        # ---------------- Phase 1: K/V projections + q(0,c0), q(1,c0) ----------
        with (
            tc.tile_pool(name="kv_ps", bufs=2, space="PSUM") as kv_ps,
            tc.tile_pool(name="tr_ps", bufs=2, space="PSUM") as tr_ps,
            tc.tile_pool(name="qp1_ps", bufs=2, space="PSUM") as qp1_ps,
            tc.tile_pool(name="p1st", bufs=2) as p1st,
        ):
            # all q pairs of chunk 0 first: frees xt chunk 0's slot for
            # chunk 3's DMA before the K/V sweep reaches it.
            for j in range(NPAIR):
                qp = qp1_ps.tile([128, SC], F32, tag="qp")
                for t in range(DT):
                    nc.tensor.matmul(
                        qp,
                        wq_sb[:, j, t, :],
                        xt_c[0][:, t, :],
                        start=(t == 0),
                        stop=(t == DT - 1),
                    )
                rope(qt_sb[j], qp, 0)

            for cs in range(NSC):
                if cs == 3:
                    # reuses xt chunk 0's buffer (q(*,c0) readers are done)
                    xt_c[3] = xtp.tile([128, DT, SC], BF16, tag="xt", name="xt_c3")
                    nc.sync.dma_start(out=xt_c[3], in_=xt[:, 3])
                xt_t = xt_c[cs]

                # KT projection + rope
                kp = kv_ps.tile([128, SC], F32, tag="kv", name="kp")
                for t in range(DT):
                    nc.tensor.matmul(
                        kp,
                        wk_sb[:, t, :],
                        xt_t[:, t, :],
                        start=(t == 0),
                        stop=(t == DT - 1),
                    )
                rope(kt_sb, kp, cs)

                # V^T projection, then transpose 128-subtiles into v_sb
                vp = kv_ps.tile([128, SC], F32, tag="kv", name="vp")
                for t in range(DT):
                    nc.tensor.matmul(
                        vp,
                        wv_sb[:, t, :],
                        xt_t[:, t, :],
                        start=(t == 0),
                        stop=(t == DT - 1),
                    )
                vt_sb = p1st.tile([128, SC], F32, tag="vtsb", bufs=2)
                nc.vector.tensor_copy(vt_sb, vp)
                for ss in range(SC // 128):
                    sk = cs * (SC // 128) + ss
                    tp = tr_ps.tile([128, 128], F32, tag="tr")
                    nc.tensor.transpose(tp, vt_sb[:, bass.ts(ss, 128)], ident_sb)
                    nc.vector.tensor_copy(v_sb[sk][:, 0:64], tp[:, 0:64])
                    nc.vector.tensor_copy(v_sb[sk][:, 65:129], tp[:, 64:128])

        # ---------------- Phase 2: attention + o_proj + remaining q-proj --------
        # Q-projection units still to produce (chunks 1-3; chunk 0 was done
        # in phase 1), in the order phase-2 units consume them: QQ[k] is
        # consumed at unit position k+4, so produce it by unit k+3.
        QQ = [(j, c) for c in range(1, NSC) for j in range(NPAIR)]

        with (
            tc.tile_pool(name="expp", bufs=6) as expp,
            tc.tile_pool(name="recp", bufs=2) as recp,
            tc.tile_pool(name="bpp", bufs=4) as bpp,
            tc.tile_pool(name="ostg", bufs=2) as ostg,
            tc.tile_pool(name="sc_ps", bufs=2, space="PSUM") as sc_ps,
            tc.tile_pool(name="av_ps", bufs=1, space="PSUM") as av_ps,
            tc.tile_pool(name="op_ps", bufs=1, space="PSUM") as op_ps,
            tc.tile_pool(name="qp_ps", bufs=1, space="PSUM") as qp_ps,
        ):
            pending = []

            def make_normalize(attn_slice, bp_slice):
                def run():
                    nc.vector.tensor_mul(attn_slice, attn_slice, bp_slice)

                return run

            def qproj_block(j, c):
                """One Q-projection unit: pair j, chunk c (16 mms + rope)."""
                qp = qp_ps.tile([128, SC], F32, tag="qp", name="qp2")
                for t in range(DT):
                    nc.tensor.matmul(
                        qp,
                        wq_sb[:, j, t, :],
                        xt_c[c][:, t, :],
                        start=(t == 0),
                        stop=(t == DT - 1),
                    )
                rope(qt_sb[j], qp, c)

            def make_oproj(st, alt_pool=False):
                """Returns per-slot emitters for o_proj of sq-subtile st
                (one dm-chunk: 4 mms + copy; final slot adds the row DMA).
                alt_pool: draw psum from qp_ps for odd chunks (drain-time
                double buffering; qp_ps is idle by then)."""
                ot = ostg.tile([128, D], BF16, tag="ostg", name="ostg")

                def chunk(mc, last):
                    def run():
                        pool = qp_ps if alt_pool and mc % 2 else op_ps
                        tag = "qp" if alt_pool and mc % 2 else "op"
                        op = pool.tile([128, SC], F32, tag=tag, name="op")
                        for t in range(ET):
                            nc.tensor.matmul(
                                op,
                                attn_sb[t][:, bass.ts(st, 128)],
                                wo_sb[:, t, bass.ts(mc, SC)],
                                start=(t == 0),
                                stop=(t == ET - 1),
                            )
                        nc.vector.tensor_copy(ot[:, bass.ts(mc, SC)], op)
                        if alt_pool:
                            # drain: per-chunk DMA on alternating queues so
                            # the final writeback tail is one chunk, not a row
                            eng = nc.sync if mc % 2 else nc.scalar
                            eng.dma_start(
                                out=out[bass.ts(st, 128), bass.ts(mc, SC)],
                                in_=ot[:, bass.ts(mc, SC)],
                            )
                        elif last:
                            nc.sync.dma_start(
                                out=out[bass.ts(st, 128), :], in_=ot
                            )

                    return run

                return [chunk(mc, mc == D // SC - 1) for mc in range(D // SC)]

            def attention(cs, j, slots, last=False):
                """Head pair j (local heads j on kv0, j+4 on kv1), sq chunk cs.

                slots: dict jj -> list of filler closures to emit at that
                iteration (PE work to absorb exp/ACT latency)."""
                sq = bass.ts(cs, SC)
                av_a = av_ps.tile([65, SC], F32, tag="ava")
                av_b = av_ps.tile([65, SC], F32, tag="avb")
                sc_t = [None, None]
                exp_t = [None] * SKT

                def scores(jj):
                    t = sc_ps.tile([128, 2 * SC], F32, tag="sc", name="sc")
                    sc_t[jj % 2] = t
                    nc.tensor.matmul(
                        t[:, 0:SC],
                        kt_sb[0:64, bass.ts(jj, 128)],
                        qt_sb[j][0:64, sq],
                        start=True,
                        stop=True,
                        tile_position=(0, 0),
                    )
                    nc.tensor.matmul(
                        t[:, SC : 2 * SC],
                        kt_sb[64:128, bass.ts(jj, 128)],
                        qt_sb[j][64:128, sq],
                        start=True,
                        stop=True,
                        tile_position=(64, 0),
                    )

                def av(t, start=False, stop=False):
                    nc.tensor.matmul(
                        av_a,
                        v_sb[t][:, 0:65],
                        exp_t[t][:, 0:SC],
                        start=start,
                        stop=stop,
                    )
                    nc.tensor.matmul(
                        av_b,
                        v_sb[t][:, 65:130],
                        exp_t[t][:, SC : 2 * SC],
                        start=start,
                        stop=stop,
                    )

                # AV runs two iterations behind its exp so the PE stream
                # rarely blocks on ACT latency.
                scores(0)
                for jj in range(SKT):
                    et = expp.tile([128, 2 * SC], BF16, tag="exp")
                    exp_t[jj] = et
                    nc.scalar.activation(et, sc_t[jj % 2], AF.Exp, scale=SCALE)
                    if jj + 1 < SKT:
                        scores(jj + 1)
                    if jj >= 2:
                        av(jj - 2, start=(jj == 2))
                    for f in slots.get(jj, ()):
                        f()
                av(SKT - 2)
                av(SKT - 1, stop=True)

                # attn copies first (release av banks for the next pair),
                # then den -> reciprocal chain, eager on DVE; the 1/den
                # partition-broadcast runs on gpsimd (idle engine); the
                # normalize multiply is deferred to early next unit.
                # reciprocal_approx_fast is a bitwise custom-DVE op and CANNOT
                # read PSUM (garbage bits) -- den must bounce through SBUF.
                # On the final pair dens go first: no next pair to unblock.
                halves = ((0, av_a), (1, av_b))
                dens = []

                def den_copies():
                    for half, avt in halves:
                        den = recp.tile([1, SC], F32, tag="den", name="den")
                        nc.vector.tensor_copy(den, avt[64:65, :])
                        dens.append(den)

                bps = []

                def rec_chain():
                    for half in (0, 1):
                        rec32 = recp.tile([1, SC], F32, tag="rec32", name="rec32")
                        nc.vector.reciprocal_approx_fast(rec32, dens[half])
                        bp = bpp.tile([128, SC], F32, tag="bp", name="bp")
                        nc.gpsimd.partition_broadcast(bp, rec32)
                        bps.append(bp)

                if last:
                    # drain is gated on the normalize: run the reciprocal +
                    # broadcast chain before the (slower) attn copies
                    den_copies()
                    rec_chain()
                for half, avt in halves:
                    attn_slice = attn_sb[j][bass.ds(64 * half, 64), sq]
                    nc.vector.tensor_copy(attn_slice, avt[0:64, :])
                if not last:
                    den_copies()
                    rec_chain()
                for half in (0, 1):
                    attn_slice = attn_sb[j][bass.ds(64 * half, 64), sq]
                    pending.append(
                        make_normalize(attn_slice, bps[half][bass.ds(64 * half, 64), :])
                    )

            # Q-unit quota per phase-2 unit: front-loaded (cs0 units have no
            # o_proj filler, so they take 2 q-units each); QQ[k] must be
            # produced no later than unit k+1 (consumed at unit k+2's scores).
            QUOTA = [2, 2, 2, 2, 1, 1, 1, 1, 0, 0, 0, 0, 0, 0, 0, 0]
            qq_next = 0
            for cs in range(NSC):
                for j in range(NPAIR):
                    pos = 4 * cs + j
                    slots = {}
                    # normalize pops for the previous unit's attn rows
                    slots[1] = [pending.pop(0) for _ in range(len(pending))]
                    # o_proj for sq-subtile of the previous chunk. In j==0
                    # units the previous pair's normalize (gpsimd broadcast +
                    # DVE mul) lands mid-unit, so start o_proj a bit later.
                    if cs > 0:
                        st = 4 * (cs - 1) + j
                        chunks = make_oproj(st)
                        op_slots = (5, 8, 11, 14) if j == 0 else (3, 6, 9, 12)
                        for sl, ch in zip(op_slots, chunks):
                            slots.setdefault(sl, []).append(ch)
                    # next Q-projection unit(s)
                    q_slots = (3, 11) if j == 0 else (5, 11)
                    for qi in range(QUOTA[pos]):
                        if qq_next < len(QQ):
                            jq, cq = QQ[qq_next]
                            qq_next += 1
                            slots.setdefault(q_slots[qi], []).append(
                                lambda jq=jq, cq=cq: qproj_block(jq, cq)
                            )
                    attention(cs, j, slots, last=(pos == 15))

            # Drain: remaining normalizes, then o_proj of the last chunk
            # (psum double-buffered across op_ps/qp_ps).
            while pending:
                pending.pop(0)()
            for j in range(NPAIR):
                for f in make_oproj(12 + j, alt_pool=True):
                    f()

    nc.compile()
    return nc


_PROGRAM = None


def _get_program():
    global _PROGRAM
    if _PROGRAM is None:
        _PROGRAM = build_program()
    return _PROGRAM


def _rope_tables():
    inv_freq = 1.0 / (ROPE_BASE ** (np.arange(0, HD, 2, dtype=np.float32) / HD))
    t = np.arange(S, dtype=np.float32)
    freqs = np.outer(t, inv_freq)  # [S, 32]
    emb = np.concatenate([freqs, freqs], axis=-1)  # [S, 64]
    return np.cos(emb).astype(np.float32), np.sin(emb).astype(np.float32)


# Feature permutation within each 64-wide head block: partition p holds
# feature PERM64[p]. Chosen so the RoPE pair (f, f+32) lands 16 partitions
# apart within one 32-partition quadrant (stream_shuffle constraint).
PERM64 = np.array(
    [p if p < 16 else p + 16 if p < 32 else p - 16 if p < 48 else p
     for p in range(64)]
)


def _host_constants():
    bf = ml_dtypes.bfloat16
    cos_t, sin_t = _rope_tables()  # [S, 64]
    idx = PERM64[np.arange(128) % HD]
    # rotate sign for the feature at partition p: rot(q)[f] = -q[f+32] for
    # f%64<32 (else +q[f-32]); with this layout that is p%32 < 16.
    sign = np.where(np.arange(128) % 32 < 16, -1.0, 1.0).astype(np.float32)
    cosd = np.ascontiguousarray(cos_t[:, idx].T).astype(bf)  # [128, S]
    sind = (np.ascontiguousarray(sin_t[:, idx].T) * sign[:, None]).astype(bf)

    ident = np.eye(128, dtype=np.float32)
    onesc = np.ones((128, 1), bf)
    return cosd, sind, ident, onesc


def _wq_pair_major(wq_c):
    """[D, E] -> [128, NPAIR, DT, 128]: pair-major so each q head-pair's
    weights are one contiguous DMA slice per partition."""
    w = wq_c.reshape(DT, 128, NPAIR, 128).transpose(1, 2, 0, 3)
    return np.ascontiguousarray(w)


def _part_major(w, n_tiles):
    """[n_tiles*128, F] -> [128, n_tiles, F]: one contiguous DMA run per
    SBUF partition (row p holds d-tiles p, 128+p, ...)."""
    f = w.shape[1]
    return np.ascontiguousarray(
        w.reshape(n_tiles, 128, f).transpose(1, 0, 2)
    )


def _core_inputs(x, Wq, Wk, Wv, Wo, consts, xt_by_batch, core):
    b, g = divmod(core, 4)
    cosd, sind, ident, onesc = consts

    wq_c = np.empty((D, E), np.float32)
    wo_c = np.empty((E, D), np.float32)
    for j in range(NPAIR):
        ha = 8 * g + j  # global head, kv-head 2g
        hb = 8 * g + j + 4  # global head, kv-head 2g+1
        # PERM64: q/k feature layout permuted per head (see _host_constants);
        # scores are invariant since q and k use the same permutation.
        wq_c[:, j * 128 : j * 128 + 64] = Wq[:, ha * HD + PERM64]
        wq_c[:, j * 128 + 64 : (j + 1) * 128] = Wq[:, hb * HD + PERM64]
        wo_c[j * 128 : j * 128 + 64, :] = Wo[ha * HD : (ha + 1) * HD, :]
        wo_c[j * 128 + 64 : (j + 1) * 128, :] = Wo[hb * HD : (hb + 1) * HD, :]
    kv0 = 2 * g * HD
    wk_c = np.concatenate(
        [Wk[:, kv0 + PERM64], Wk[:, kv0 + HD + PERM64]], axis=1
    )
    wv_c = np.ascontiguousarray(Wv[:, kv0 : kv0 + KVW])

    bf = ml_dtypes.bfloat16
    return {
        "xt": xt_by_batch[b],
        "wq": _wq_pair_major(wq_c.astype(bf)),
        "wk": _part_major(wk_c.astype(bf), DT),
        "wv": _part_major(wv_c.astype(bf), DT),
        "wo": _part_major(wo_c.astype(bf), ET),
        "cosd": cosd,
        "sind": sind,
        "ident": ident,
        "onesc": onesc,
    }


def make_in_maps(x, Wq, Wk, Wv, Wo):
    consts = _host_constants()
    # xt host layout [128, NSC, DT, SC]: partition-major with the s-chunk
    # outermost below the partition so each chunk is one contiguous run per
    # partition. xt[p, c, t, s] = x[b].T[t*128+p, c*SC+s].
    xt_by_batch = []
    for b in range(B):
        xtb = np.ascontiguousarray(x[b].T).astype(ml_dtypes.bfloat16)  # [D, S]
        xtb = xtb.reshape(DT, 128, NSC, SC).transpose(1, 2, 0, 3)
        xt_by_batch.append(np.ascontiguousarray(xtb))
    return [
        _core_inputs(x, Wq, Wk, Wv, Wo, consts, xt_by_batch, c)
        for c in range(N_CORES)
    ]


def kernel(x, Wq, Wk, Wv, Wo, _trace=False, _trace_kwargs=None):
    x = np.asarray(x, np.float32)
    Wq = np.asarray(Wq, np.float32)
    Wk = np.asarray(Wk, np.float32)
    Wv = np.asarray(Wv, np.float32)
    Wo = np.asarray(Wo, np.float32)

    nc = _get_program()
    in_maps = make_in_maps(x, Wq, Wk, Wv, Wo)
    res = bass_utils.run_bass_kernel_spmd(
        nc,
        in_maps,
        core_ids=list(range(N_CORES)),
        trace=_trace,
        **(_trace_kwargs or {}),
    )
    outs = [np.asarray(r["out"], ml_dtypes.bfloat16).astype(np.float32)
            for r in res.results]
    full = np.empty((B, S, D), np.float32)
    for b in range(B):
        full[b] = outs[4 * b] + outs[4 * b + 1] + outs[4 * b + 2] + outs[4 * b + 3]
    if _trace:
        return full, res
    return full


# revision 30
# speedup vs baseline: 1.0026x; 1.0026x over previous
"""GQA attention block (Wq/Wk/Wv -> RoPE -> softmax(QK^T)V -> Wo) on 8 Trainium2
NeuronCores.

Sharding (tensor-parallel per the head-sharding scheme):
  core c in 0..7: batch b = c // 4, head-group g = c % 4.
  Each core owns 8 q-heads (global 8g..8g+7) and 2 kv-heads (2g, 2g+1) of one
  batch element, computes its slice of q/k/v projections, RoPE, attention, and
  a partial o_proj (Wo rows for its heads). The all-reduce after o_proj is the
  host-side unshard: out[b] = sum of the 4 partial outputs of batch b.

On-device layout (per core), everything feature-on-partitions ("transposed"):
  xt    [D=2048, S=2048]   x^T for this batch
  QT    [E=512,  S]        q^T; partition-tile j holds head pair (j, j+4):
                           local head j (kv0) on partitions 0:64, head j+4
                           (kv1) on partitions 64:128. Wq columns are permuted
                           on the host to produce this layout directly.
  KT    [128, S]           k^T; kv0 on partitions 0:64, kv1 on 64:128.
  V     [S, 130] as 16 tiles [128, 130]: cols 0:64 v(kv0), col 64 ones,
                           cols 65:129 v(kv1), col 129 ones  (v_aug).
  scores^T per head: [sk, sq] so exp is ACT psum->sbuf and the attn@v
  contraction (over sk) uses v_aug as the stationary operand; row 64 of the
  attn@v output is the softmax denominator (ones column trick).

Schedule (v2): phase 1 only computes q pairs 0,1 of chunk 0 and all K/V;
the remaining 14 Q-projection units run as PE filler inside the phase-2
attention pipeline (which is otherwise exp/ACT-gated), alongside the o_proj
chunks. The softmax 1/den broadcast runs on the idle GpSimd engine instead
of PE matmuls. Output is written bf16 (summed fp32 on host).
"""

import sys

if "/opt/trn_rl_repo" not in sys.path:
    sys.path.insert(0, "/opt/trn_rl_repo")

from contextlib import ExitStack

import numpy as np
import ml_dtypes

import concourse.bass as bass  # noqa: F401  (engine types via nc)
import concourse.tile as tile
from concourse import bacc, bass_utils, mybir

F32 = mybir.dt.float32
F32R = mybir.dt.float32r
BF16 = mybir.dt.bfloat16
AF = mybir.ActivationFunctionType

# Problem constants (hardcoded per harness contract)
B = 2
S = 2048  # sequence length
D = 2048  # d_model
N_HEADS = 32
N_KV = 8
HD = 64  # head dim
ROPE_BASE = 500000.0
N_CORES = 8

# Per-core derived
NQ = N_HEADS // 4  # 8 local q heads (4 head-groups)
E = NQ * HD  # 512 local q features
NPAIR = NQ // 2  # 4 head pairs / e-tiles
KVW = 2 * HD  # 128 local kv features
SC = 512  # s-chunk (projection + sq chunk)
NSC = S // SC  # 4
DT = D // 128  # 16 d-tiles
SKT = S // 128  # 16 sk tiles
ET = E // 128  # 4 e-tiles
SCALE = 1.0 / float(np.sqrt(HD))

SHUF_MASK = [(i + 16) % 32 for i in range(32)]


def build_program():
    nc = bacc.Bacc(
        "TRN2", target_bir_lowering=False, debug=False, enable_asserts=False
    )

    # All large inputs are pre-arranged on the host so each DMA reads one
    # contiguous run per partition (128 fat descriptors instead of 2048
    # small ones; descriptor generation on the issuing queue is the startup
    # bottleneck otherwise).
    xt = nc.dram_tensor("xt", [128, NSC, DT, SC], BF16, kind="ExternalInput").ap()
    wq = nc.dram_tensor("wq", [128, NPAIR, DT, 128], BF16, kind="ExternalInput").ap()
    wk = nc.dram_tensor("wk", [128, DT, KVW], BF16, kind="ExternalInput").ap()
    wv = nc.dram_tensor("wv", [128, DT, KVW], BF16, kind="ExternalInput").ap()
    wo = nc.dram_tensor("wo", [128, ET, D], BF16, kind="ExternalInput").ap()
    cosd = nc.dram_tensor("cosd", [128, S], BF16, kind="ExternalInput").ap()
    sind = nc.dram_tensor("sind", [128, S], BF16, kind="ExternalInput").ap()
    ident = nc.dram_tensor("ident", [128, 128], F32, kind="ExternalInput").ap()
    onesc = nc.dram_tensor("onesc", [128, 1], BF16, kind="ExternalInput").ap()
    out = nc.dram_tensor("out", [S, D], BF16, kind="ExternalOutput").ap()

    with tile.TileContext(nc) as tc, ExitStack() as ctx:
        persist = ctx.enter_context(tc.tile_pool(name="persist", bufs=1))
        xtp = ctx.enter_context(tc.tile_pool(name="xtp", bufs=3))
        ropec = ctx.enter_context(tc.tile_pool(name="ropec", bufs=1))
        ropet = ctx.enter_context(tc.tile_pool(name="ropet", bufs=2))

        # Persistent SBUF state
        qt_sb = [persist.tile([128, S], BF16, tag=f"qt{j}", name=f"qt{j}") for j in range(NPAIR)]
        kt_sb = persist.tile([128, S], BF16, tag="kt")
        v_sb = [persist.tile([128, 130], BF16, tag=f"v{j}", name=f"v{j}") for j in range(SKT)]
        attn_sb = [persist.tile([128, S], BF16, tag=f"at{j}", name=f"at{j}") for j in range(NPAIR)]
        onesc_sb = persist.tile([128, 1], BF16, tag="onesc")

        wq_sb = persist.tile([128, NPAIR, DT, 128], BF16, tag="wq")
        wk_sb = persist.tile([128, DT, KVW], BF16, tag="wk")
        wv_sb = persist.tile([128, DT, KVW], BF16, tag="wv")
        wo_sb = persist.tile([128, ET, D], BF16, tag="wo")

        # ---- input DMAs, spread across hw queues in first-use order ----
        # Per-queue DMA bandwidth is ~170GB/s with ~8us ring spin-up, so the
        # startup-critical loads (wq+xt0 for q-proj, then wk/wv/rope tables)
        # go on four separate queues. cos/sin load in per-chunk slices so
        # chunk-0 rope isn't gated on the full 4MB.
        xt_c = [None] * NSC
        for c in range(3):
            xt_c[c] = xtp.tile([128, DT, SC], BF16, tag="xt", name=f"xt_c{c}")
        # sync queue: xt chunk 0 in 4 t-group slices (q-proj mms start as
        # soon as the first slice + wq pair 0 land), then chunks 1 and (mid
        # phase-1) 3.
        for i in range(4):
            nc.sync.dma_start(
                out=xt_c[0][:, bass.ds(4 * i, 4), :],
                in_=xt[:, 0, bass.ds(4 * i, 4), :],
            )
        nc.sync.dma_start(out=xt_c[1], in_=xt[:, 1])
        # scalar queue: wq in pair slices (paced with the q-proj units),
        # small weights, xt chunk 2, then late rope tables.
        cos_sb = ropec.tile([128, S], BF16, tag="cos")
        sin_sb = ropec.tile([128, S], BF16, tag="sin")
        for j in range(NPAIR):
            nc.scalar.dma_start(out=wq_sb[:, j], in_=wq[:, j])
        nc.scalar.dma_start(out=wk_sb, in_=wk)
        nc.scalar.dma_start(out=wv_sb, in_=wv)
        ident_sb = ropec.tile([128, 128], F32, tag="ident")
        nc.scalar.dma_start(out=ident_sb, in_=ident)
        nc.scalar.dma_start(out=xt_c[2], in_=xt[:, 2])
        for c in range(2, NSC):
            nc.scalar.dma_start(out=cos_sb[:, bass.ts(c, SC)], in_=cosd[:, bass.ts(c, SC)])
            nc.scalar.dma_start(out=sin_sb[:, bass.ts(c, SC)], in_=sind[:, bass.ts(c, SC)])
        # gpsimd SWDGE: early rope tables (q/K rope of chunks 0-1 needs
        # them by ~25us) + wo (needed ~+150us).
        nc.gpsimd.dma_start(out=onesc_sb, in_=onesc)
        for c in (0, 1):
            nc.gpsimd.dma_start(out=cos_sb[:, bass.ts(c, SC)], in_=cosd[:, bass.ts(c, SC)])
            nc.gpsimd.dma_start(out=sin_sb[:, bass.ts(c, SC)], in_=sind[:, bass.ts(c, SC)])
        nc.gpsimd.dma_start(out=wo_sb, in_=wo)

        # v_aug ones columns are constant: prefill once on gpsimd.
        for t in range(SKT):
            nc.gpsimd.tensor_copy(v_sb[t][:, 64:65], onesc_sb)
            nc.gpsimd.tensor_copy(v_sb[t][:, 129:130], onesc_sb)

        def rope(dst, src_ps, cs):
            """dst[:, cs*SC:+SC] = src_ps*cos + shuffle(src)*sin_signed.

            Features are laid out (host-side permutation) so the RoPE
            rotate pairing is a +-16 swap within each 32-partition
            quadrant; the rotate sign is folded into sind."""
            sl = bass.ts(cs, SC)
            raw = ropet.tile([128, SC], F32R, tag="raw", name="raw", bufs=3)
            nc.vector.tensor_copy(raw, src_ps)
            rp = ropet.tile([128, SC], F32, tag="shuf", name="shuf", bufs=3)
            nc.vector.stream_shuffle(rp, raw, SHUF_MASK)
            tcos = ropet.tile([128, SC], F32, tag="tmp", name="tcos", bufs=4)
            nc.vector.tensor_mul(tcos, raw, cos_sb[:, sl])
            tsin = ropet.tile([128, SC], F32, tag="tmp", name="tsin", bufs=4)
            nc.vector.tensor_mul(tsin, rp, sin_sb[:, sl])
            nc.vector.tensor_add(dst[:, sl], tcos, tsin)

        # ---------------- Phase 1: K/V projections + q(0,c0), q(1,c0) ----------
        with (
            tc.tile_pool(name="kv_ps", bufs=2, space="PSUM") as kv_ps,
            tc.tile_pool(name="tr_ps", bufs=2, space="PSUM") as tr_ps,
            tc.tile_pool(name="qp1_ps", bufs=2, space="PSUM") as qp1_ps,
            tc.tile_pool(name="p1st", bufs=2) as p1st,
        ):
            # all q pairs of chunk 0 first: frees xt chunk 0's slot for
            # chunk 3's DMA before the K/V sweep reaches it.
            for j in range(NPAIR):
                qp = qp1_ps.tile([128, SC], F32, tag="qp")
                for t in range(DT):
                    nc.tensor.matmul(
                        qp,
                        wq_sb[:, j, t, :],
                        xt_c[0][:, t, :],
                        start=(t == 0),
                        stop=(t == DT - 1),
                    )
                rope(qt_sb[j], qp, 0)

            for cs in range(NSC):
                if cs == 3:
                    # reuses xt chunk 0's buffer (q(*,c0) readers are done)
                    xt_c[3] = xtp.tile([128, DT, SC], BF16, tag="xt", name="xt_c3")
                    nc.sync.dma_start(out=xt_c[3], in_=xt[:, 3])
                xt_t = xt_c[cs]

                # KT projection + rope
                kp = kv_ps.tile([128, SC], F32, tag="kv", name="kp")
                for t in range(DT):
                    nc.tensor.matmul(
                        kp,
                        wk_sb[:, t, :],
                        xt_t[:, t, :],
                        start=(t == 0),
                        stop=(t == DT - 1),
                    )
                rope(kt_sb, kp, cs)

                # V^T projection, then transpose 128-subtiles into v_sb
                vp = kv_ps.tile([128, SC], F32, tag="kv", name="vp")
                for t in range(DT):
                    nc.tensor.matmul(
                        vp,
                        wv_sb[:, t, :],
                        xt_t[:, t, :],
                        start=(t == 0),
                        stop=(t == DT - 1),
                    )
                vt_sb = p1st.tile([128, SC], F32, tag="vtsb", bufs=2)
                nc.vector.tensor_copy(vt_sb, vp)
                for ss in range(SC // 128):
                    sk = cs * (SC // 128) + ss
                    tp = tr_ps.tile([128, 128], F32, tag="tr")
                    nc.tensor.transpose(tp, vt_sb[:, bass.ts(ss, 128)], ident_sb)
                    nc.vector.tensor_copy(v_sb[sk][:, 0:64], tp[:, 0:64])
                    nc.vector.tensor_copy(v_sb[sk][:, 65:129], tp[:, 64:128])

        # ---------------- Phase 2: attention + o_proj + remaining q-proj --------
        # Q-projection units still to produce (chunks 1-3; chunk 0 was done
        # in phase 1), in the order phase-2 units consume them: QQ[k] is
        # consumed at unit position k+4, so produce it by unit k+3.
        QQ = [(j, c) for c in range(1, NSC) for j in range(NPAIR)]

        with (
            tc.tile_pool(name="expp", bufs=6) as expp,
            tc.tile_pool(name="recp", bufs=2) as recp,
            tc.tile_pool(name="bpp", bufs=4) as bpp,
            tc.tile_pool(name="ostg", bufs=2) as ostg,
            tc.tile_pool(name="sc_ps", bufs=2, space="PSUM") as sc_ps,
            tc.tile_pool(name="av_ps", bufs=1, space="PSUM") as av_ps,
            tc.tile_pool(name="op_ps", bufs=1, space="PSUM") as op_ps,
            tc.tile_pool(name="qp_ps", bufs=1, space="PSUM") as qp_ps,
        ):
            pending = []

            def make_normalize(attn_slice, bp_slice):
                def run():
                    nc.vector.tensor_mul(attn_slice, attn_slice, bp_slice)

                return run

            def qproj_block(j, c):
                """One Q-projection unit: pair j, chunk c (16 mms + rope)."""
                qp = qp_ps.tile([128, SC], F32, tag="qp", name="qp2")
                for t in range(DT):
                    nc.tensor.matmul(
                        qp,
                        wq_sb[:, j, t, :],
                        xt_c[c][:, t, :],
                        start=(t == 0),
                        stop=(t == DT - 1),
                    )
                rope(qt_sb[j], qp, c)

            def make_oproj(st, alt_pool=False):
                """Returns per-slot emitters for o_proj of sq-subtile st
                (one dm-chunk: 4 mms + copy; final slot adds the row DMA).
                alt_pool: draw psum from qp_ps for odd chunks (drain-time
                double buffering; qp_ps is idle by then)."""
                ot = ostg.tile([128, D], BF16, tag="ostg", name="ostg")

                def chunk(mc, last):
                    def run():
                        pool = qp_ps if alt_pool and mc % 2 else op_ps
                        tag = "qp" if alt_pool and mc % 2 else "op"
                        op = pool.tile([128, SC], F32, tag=tag, name="op")
                        for t in range(ET):
                            nc.tensor.matmul(
                                op,
                                attn_sb[t][:, bass.ts(st, 128)],
                                wo_sb[:, t, bass.ts(mc, SC)],
                                start=(t == 0),
                                stop=(t == ET - 1),
                            )
                        nc.vector.tensor_copy(ot[:, bass.ts(mc, SC)], op)
                        if alt_pool:
                            # drain: per-chunk DMA on alternating queues so
                            # the final writeback tail is one chunk, not a row
                            eng = nc.sync if mc % 2 else nc.scalar
                            eng.dma_start(
                                out=out[bass.ts(st, 128), bass.ts(mc, SC)],
                                in_=ot[:, bass.ts(mc, SC)],
                            )
                        elif last:
                            nc.sync.dma_start(
                                out=out[bass.ts(st, 128), :], in_=ot
                            )

                    return run

                return [chunk(mc, mc == D // SC - 1) for mc in range(D // SC)]

            def attention(cs, j, slots, last=False):
                """Head pair j (local heads j on kv0, j+4 on kv1), sq chunk cs.

                slots: dict jj -> list of filler closures to emit at that
                iteration (PE work to absorb exp/ACT latency)."""
                sq = bass.ts(cs, SC)
                av_a = av_ps.tile([65, SC], F32, tag="ava")
                av_b = av_ps.tile([65, SC], F32, tag="avb")
                sc_t = [None, None]
                exp_t = [None] * SKT

                def scores(jj):
                    t = sc_ps.tile([128, 2 * SC], F32, tag="sc", name="sc")
                    sc_t[jj % 2] = t
                    nc.tensor.matmul(
                        t[:, 0:SC],
                        kt_sb[0:64, bass.ts(jj, 128)],
                        qt_sb[j][0:64, sq],
                        start=True,
                        stop=True,
                        tile_position=(0, 0),
                    )
                    nc.tensor.matmul(
                        t[:, SC : 2 * SC],
                        kt_sb[64:128, bass.ts(jj, 128)],
                        qt_sb[j][64:128, sq],
                        start=True,
                        stop=True,
                        tile_position=(64, 0),
                    )

                def av(t, start=False, stop=False):
                    nc.tensor.matmul(
                        av_a,
                        v_sb[t][:, 0:65],
                        exp_t[t][:, 0:SC],
                        start=start,
                        stop=stop,
                    )
                    nc.tensor.matmul(
                        av_b,
                        v_sb[t][:, 65:130],
                        exp_t[t][:, SC : 2 * SC],
                        start=start,
                        stop=stop,
                    )

                # AV runs two iterations behind its exp so the PE stream
                # rarely blocks on ACT latency.
                scores(0)
                for jj in range(SKT):
                    et = expp.tile([128, 2 * SC], BF16, tag="exp")
                    exp_t[jj] = et
                    nc.scalar.activation(et, sc_t[jj % 2], AF.Exp, scale=SCALE)
                    if jj + 1 < SKT:
                        scores(jj + 1)
                    if jj >= 2:
                        av(jj - 2, start=(jj == 2))
                    for f in slots.get(jj, ()):
                        f()
                av(SKT - 2)
                av(SKT - 1, stop=True)

                # attn copies first (release av banks for the next pair),
                # then den -> reciprocal chain, eager on DVE; the 1/den
                # partition-broadcast runs on gpsimd (idle engine); the
                # normalize multiply is deferred to early next unit.
                # reciprocal_approx_fast is a bitwise custom-DVE op and CANNOT
                # read PSUM (garbage bits) -- den must bounce through SBUF.
                # On the final pair dens go first: no next pair to unblock.
                halves = ((0, av_a), (1, av_b))
                dens = []

                def den_copies():
                    for half, avt in halves:
                        den = recp.tile([1, SC], F32, tag="den", name="den")
                        nc.vector.tensor_copy(den, avt[64:65, :])
                        dens.append(den)

                bps = []

                def rec_chain():
                    for half in (0, 1):
                        rec32 = recp.tile([1, SC], F32, tag="rec32", name="rec32")
                        nc.vector.reciprocal_approx_fast(rec32, dens[half])
                        bp = bpp.tile([128, SC], F32, tag="bp", name="bp")
                        nc.gpsimd.partition_broadcast(bp, rec32)
                        bps.append(bp)

                if last:
                    # drain is gated on the normalize: run the reciprocal +
                    # broadcast chain before the (slower) attn copies
                    den_copies()
                    rec_chain()
                for half, avt in halves:
                    attn_slice = attn_sb[j][bass.ds(64 * half, 64), sq]
                    nc.vector.tensor_copy(attn_slice, avt[0:64, :])
                if not last:
                    den_copies()
                    rec_chain()
                for half in (0, 1):
                    attn_slice = attn_sb[j][bass.ds(64 * half, 64), sq]
                    pending.append(
                        make_normalize(attn_slice, bps[half][bass.ds(64 * half, 64), :])
                    )

            # Q-unit quota per phase-2 unit: front-loaded (cs0 units have no
            # o_proj filler, so they take 2 q-units each); QQ[k] must be
            # produced no later than unit k+1 (consumed at unit k+2's scores).
            QUOTA = [2, 2, 2, 2, 1, 1, 1, 1, 0, 0, 0, 0, 0, 0, 0, 0]
            qq_next = 0
            for cs in range(NSC):
                for j in range(NPAIR):
                    pos = 4 * cs + j
                    slots = {}
                    # normalize pops for the previous unit's attn rows
                    slots[1] = [pending.pop(0) for _ in range(len(pending))]
                    # o_proj for sq-subtile of the previous chunk. In j==0
                    # units the previous pair's normalize (gpsimd broadcast +
                    # DVE mul) lands mid-unit, so start o_proj a bit later.
                    if cs > 0:
                        st = 4 * (cs - 1) + j
                        chunks = make_oproj(st)
                        op_slots = (5, 8, 11, 14) if j == 0 else (3, 6, 9, 12)
                        for sl, ch in zip(op_slots, chunks):
                            slots.setdefault(sl, []).append(ch)
                    # next Q-projection unit(s)
                    q_slots = (3, 11) if j == 0 else (5, 11)
                    for qi in range(QUOTA[pos]):
                        if qq_next < len(QQ):
                            jq, cq = QQ[qq_next]
                            qq_next += 1
                            slots.setdefault(q_slots[qi], []).append(
                                lambda jq=jq, cq=cq: qproj_block(jq, cq)
                            )
                    attention(cs, j, slots, last=(pos == 15))

            # Drain: remaining normalizes, then o_proj of the last chunk
            # (psum double-buffered across op_ps/qp_ps).
            while pending:
                pending.pop(0)()
            for j in range(NPAIR):
                for f in make_oproj(12 + j, alt_pool=True):
                    f()

    nc.compile()
    return nc


_PROGRAM = None


def _get_program():
    global _PROGRAM
    if _PROGRAM is None:
        _PROGRAM = build_program()
    return _PROGRAM


def _rope_tables():
    inv_freq = 1.0 / (ROPE_BASE ** (np.arange(0, HD, 2, dtype=np.float32) / HD))
    t = np.arange(S, dtype=np.float32)
    freqs = np.outer(t, inv_freq)  # [S, 32]
    emb = np.concatenate([freqs, freqs], axis=-1)  # [S, 64]
    return np.cos(emb).astype(np.float32), np.sin(emb).astype(np.float32)


# Feature permutation within each 64-wide head block: partition p holds
# feature PERM64[p]. Chosen so the RoPE pair (f, f+32) lands 16 partitions
# apart within one 32-partition quadrant (stream_shuffle constraint).
PERM64 = np.array(
    [p if p < 16 else p + 16 if p < 32 else p - 16 if p < 48 else p
     for p in range(64)]
)


def _host_constants():
    bf = ml_dtypes.bfloat16
    cos_t, sin_t = _rope_tables()  # [S, 64]
    idx = PERM64[np.arange(128) % HD]
    # rotate sign for the feature at partition p: rot(q)[f] = -q[f+32] for
    # f%64<32 (else +q[f-32]); with this layout that is p%32 < 16.
    sign = np.where(np.arange(128) % 32 < 16, -1.0, 1.0).astype(np.float32)
    cosd = np.ascontiguousarray(cos_t[:, idx].T).astype(bf)  # [128, S]
    sind = (np.ascontiguousarray(sin_t[:, idx].T) * sign[:, None]).astype(bf)

    ident = np.eye(128, dtype=np.float32)
    onesc = np.ones((128, 1), bf)
    return cosd, sind, ident, onesc


def _wq_pair_major(wq_c):
    """[D, E] -> [128, NPAIR, DT, 128]: pair-major so each q head-pair's
    weights are one contiguous DMA slice per partition."""
    w = wq_c.reshape(DT, 128, NPAIR, 128).transpose(1, 2, 0, 3)
    return np.ascontiguousarray(w)


def _part_major(w, n_tiles):
    """[n_tiles*128, F] -> [128, n_tiles, F]: one contiguous DMA run per
    SBUF partition (row p holds d-tiles p, 128+p, ...)."""
    f = w.shape[1]
    return np.ascontiguousarray(
        w.reshape(n_tiles, 128, f).transpose(1, 0, 2)
    )


def _core_inputs(x, Wq, Wk, Wv, Wo, consts, xt_by_batch, core):
    b, g = divmod(core, 4)
    cosd, sind, ident, onesc = consts

    wq_c = np.empty((D, E), np.float32)
    wo_c = np.empty((E, D), np.float32)
    for j in range(NPAIR):
        ha = 8 * g + j  # global head, kv-head 2g
        hb = 8 * g + j + 4  # global head, kv-head 2g+1
        # PERM64: q/k feature layout permuted per head (see _host_constants);
        # scores are invariant since q and k use the same permutation.
        wq_c[:, j * 128 : j * 128 + 64] = Wq[:, ha * HD + PERM64]
        wq_c[:, j * 128 + 64 : (j + 1) * 128] = Wq[:, hb * HD + PERM64]
        wo_c[j * 128 : j * 128 + 64, :] = Wo[ha * HD : (ha + 1) * HD, :]
        wo_c[j * 128 + 64 : (j + 1) * 128, :] = Wo[hb * HD : (hb + 1) * HD, :]
    kv0 = 2 * g * HD
    wk_c = np.concatenate(
        [Wk[:, kv0 + PERM64], Wk[:, kv0 + HD + PERM64]], axis=1
    )
    wv_c = np.ascontiguousarray(Wv[:, kv0 : kv0 + KVW])

    bf = ml_dtypes.bfloat16
    return {
        "xt": xt_by_batch[b],
        "wq": _wq_pair_major(wq_c.astype(bf)),
        "wk": _part_major(wk_c.astype(bf), DT),
        "wv": _part_major(wv_c.astype(bf), DT),
        "wo": _part_major(wo_c.astype(bf), ET),
        "cosd": cosd,
        "sind": sind,
        "ident": ident,
        "onesc": onesc,
    }


def make_in_maps(x, Wq, Wk, Wv, Wo):
    consts = _host_constants()
    # xt host layout [128, NSC, DT, SC]: partition-major with the s-chunk
    # outermost below the partition so each chunk is one contiguous run per
    # partition. xt[p, c, t, s] = x[b].T[t*128+p, c*SC+s].
    xt_by_batch = []
    for b in range(B):
        xtb = np.ascontiguousarray(x[b].T).astype(ml_dtypes.bfloat16)  # [D, S]
        xtb = xtb.reshape(DT, 128, NSC, SC).transpose(1, 2, 0, 3)
        xt_by_batch.append(np.ascontiguousarray(xtb))
    return [
        _core_inputs(x, Wq, Wk, Wv, Wo, consts, xt_by_batch, c)
        for c in range(N_CORES)
    ]


def kernel(x, Wq, Wk, Wv, Wo, _trace=False, _trace_kwargs=None):
    x = np.asarray(x, np.float32)
    Wq = np.asarray(Wq, np.float32)
    Wk = np.asarray(Wk, np.float32)
    Wv = np.asarray(Wv, np.float32)
    Wo = np.asarray(Wo, np.float32)

    nc = _get_program()
    in_maps = make_in_maps(x, Wq, Wk, Wv, Wo)
    res = bass_utils.run_bass_kernel_spmd(
        nc,
        in_maps,
        core_ids=list(range(N_CORES)),
        trace=_trace,
        **(_trace_kwargs or {}),
    )
    outs = [np.asarray(r["out"], ml_dtypes.bfloat16).astype(np.float32)
            for r in res.results]
    full = np.empty((B, S, D), np.float32)
    for b in range(B):
        full[b] = outs[4 * b] + outs[4 * b + 1] + outs[4 * b + 2] + outs[4 * b + 3]
    if _trace:
        return full, res
    return full
